# revision 17
# baseline (speedup 1.0000x reference)
"""LoOP (Local Outlier Probability) kernel for 8 TRN2 NeuronCores.

kernel(X, train_points) computes the reference nn_LoOP forward pass:
brute-force 20-NN of X over train_points, the 20-NN of each neighbor,
pdist ratios, and max(erf(lof/sqrt(2)), 0) -- distributed over 8 cores
(row-sharded train_points), with all compute on-device.

Distance trick: with u_t = t - X stashed transposed (bf16) and
unb_k = nb_k - X, the score s = 2*u_t.unb_k - ||u_t||^2 equals
||nb_k - X||^2 - ||t - nb_k||^2, so per-neighbor nearest rows are
argmax(s) and exact-enough distances are d0x_k - s (d0x_k exact f32).
No per-candidate gather/re-square is needed for the second kNN.
"""

import sys
import types
from contextlib import ExitStack

import numpy as np

import bass_rust
import concourse.bass as bass
import concourse.mybir as mybir
import concourse.tile as tile
from concourse.masks import make_identity
from concourse.tile import TileContext
from concourse.vector_clock import ScopedClock


# ---------------------------------------------------------------------------
# Toolchain workarounds: this walrus build accepts at most ONE sync wait per
# instruction (two for EventSemaphore), and the Tile kernel-tail drain
# collects one wait per outstanding sem domain. Split both.
# ---------------------------------------------------------------------------
def _split_multi_waits(nc):
    """This walrus build accepts at most ONE sync wait per instruction
    (two for EventSemaphore). Tile attaches as many waits as deps require.
    Rewrite: keep the first wait on the instruction, hoist extras onto
    same-engine NOPs inserted immediately before it."""
    edits = []
    for f in nc.m.functions:
        for bb in f.blocks:
            edits.append((bb, list(bb.instructions)))
    new_lists = []
    for bb, insts in edits:
        new = []
        changed = False
        for inst in insts:
            si = inst.sync_info
            cap = 2 if isinstance(inst, bass_rust.InstEventSemaphore) else 1
            if si is not None and si.on_wait and len(si.on_wait) > cap:
                waits = list(si.on_wait)
                for w in waits[cap:]:
                    nop = nc.engines[inst.engine].nop(nofuse=True).ins
                    nop.sync_info = bass_rust.SyncInfo(on_wait=[w],
                                                       on_update=[])
                    new.append(nop)
                inst.sync_info = bass_rust.SyncInfo(
                    on_wait=waits[:cap], on_update=list(si.on_update or []))
                changed = True
            new.append(inst)
        new_lists.append((bb, new, changed))
    for bb, new, changed in new_lists:
        if changed:
            bb.instructions = new


def _patched_drain_and_barrier(self, tick_clock, wait_clock):
    nc = self.nc
    _split_multi_waits(nc)
    drain_inst = nc.sync.drain()
    wait_clock.add_sem_waits(
        drain_inst.ins, ScopedClock({None: tick_clock.global_clock})
    )
    si = drain_inst.ins.sync_info
    if si is not None and si.on_wait and len(si.on_wait) > 1:
        waits = list(si.on_wait)
        upd = list(si.on_update or [])
        drain_inst.ins.sync_info = bass_rust.SyncInfo(
            on_wait=[waits[0]], on_update=upd
        )
        for w in waits[1:]:
            extra = nc.sync.drain()
            extra.ins.sync_info = bass_rust.SyncInfo(on_wait=[w], on_update=[])

    nc.all_engine_barrier()
    assert self.sems is not None
    popped = nc._tile_sem_poison_stack.pop()
    assert popped is self._sem_poison
    nc.clear_and_free_semaphores(list(self.sems.allocated().values()))
    nc.all_engine_barrier()


def install():
    TileContext._drain_and_barrier = _patched_drain_and_barrier
    try:
        _install_ntff_hook()
    except Exception:
        pass  # profiling hook is optional


def _install_ntff_hook():
    if "antenv.axon_hooks" in sys.modules:
        return
    mod = types.ModuleType("antenv.axon_hooks")
    state = {"hook": None}
    mod.set_axon_ntff_profile_hook = lambda h: state.__setitem__("hook", h)
    mod.get_axon_ntff_profile_hook = lambda: state["hook"]
    sys.modules["antenv.axon_hooks"] = mod
    import antenv

    antenv.axon_hooks = mod
    from trn_agent_boot.trn_boot import _ntff_profile_via_ctypes

    hook = _ntff_profile_via_ctypes("/opt/axon/libaxon_pjrt.so")
    if hook is not None:
        mod.set_axon_ntff_profile_hook(hook)




install()


F32 = mybir.dt.float32
BF16 = mybir.dt.bfloat16
U32 = mybir.dt.uint32
AF = mybir.ActivationFunctionType
ALU = mybir.AluOpType

NC_N = 8          # cores
D = 512           # feature dim
K = 20            # neighbors
NT = 98           # tiles per core
NLOC = NT * 128   # 12544 rows per core (padded)
NPAD = NC_N * NLOC
PADV = 1.0e4      # padding row fill value
NEG = -3.0e38

SQ2I = 0.7071067811865476
TPI = 1.1283791670955126  # 2/sqrt(pi)


def _rounds_topk_vi(nc, work, vals, pos, n_rounds=3):
    """max/match_replace rounds on `work` [P, F]; writes descending values
    into vals [P, 8*n] and positions into pos (uint32)."""
    for r in range(n_rounds):
        v8 = vals[:, 8 * r:8 * r + 8]
        nc.vector.max_with_indices(out_max=v8,
                                   out_indices=pos[:, 8 * r:8 * r + 8],
                                   in_=work)
        if r < n_rounds - 1:
            nc.vector.match_replace(out=work, in_to_replace=v8,
                                    in_values=work, imm_value=NEG)


def _rounds_topk_v(nc, work, vals, n_rounds=3):
    """values-only top-8*n rounds (no index extraction)."""
    for r in range(n_rounds):
        v8 = vals[:, 8 * r:8 * r + 8]
        nc.vector.max(out=v8, in_=work)
        if r < n_rounds - 1:
            nc.vector.match_replace(out=work, in_to_replace=v8,
                                    in_values=work, imm_value=NEG)


def build(debug=False, stage=99):
    nc = bass.Bass()
    tp = nc.declare_dram_parameter("tp", [NLOC, D], BF16, isOutput=False)
    tpf = nc.declare_dram_parameter("tpf", [NLOC, D], F32, isOutput=False)
    x_in = nc.declare_dram_parameter("x", [1, D], F32, isOutput=False)
    out = nc.declare_dram_parameter("out", [1, 1], F32, isOutput=True)
    if debug:
        dbg_d0 = nc.declare_dram_parameter("dbg_d0", [128, NT], F32, isOutput=True)
        dbg_nl24 = nc.declare_dram_parameter("dbg_nl24", [24, 1], U32, isOutput=True)
        dbg_d0x = nc.declare_dram_parameter("dbg_d0x", [24, 1], F32, isOutput=True)
        dbg_nb = nc.declare_dram_parameter("dbg_nb", [K, D], F32, isOutput=True)
        dbg_s2 = nc.declare_dram_parameter("dbg_s2", [K, 1], F32, isOutput=True)

    with tile.TileContext(nc) as tc, ExitStack() as ctx:
        # ---- pools ----
        consts = ctx.enter_context(tc.tile_pool(name="consts", bufs=1))
        big = ctx.enter_context(tc.tile_pool(name="big", bufs=1))
        ld = ctx.enter_context(tc.tile_pool(name="ld", bufs=6))
        bfp = ctx.enter_context(tc.tile_pool(name="bfp", bufs=4))
        scrp = ctx.enter_context(tc.tile_pool(name="scrp", bufs=3))
        small = ctx.enter_context(tc.tile_pool(name="small", bufs=1))
        psum_t = ctx.enter_context(tc.tile_pool(name="psum_t", bufs=3, space="PSUM"))
        psum_c = ctx.enter_context(tc.tile_pool(name="psum_c", bufs=3, space="PSUM"))
        psum_s = ctx.enter_context(tc.tile_pool(name="psum_s", bufs=1, space="PSUM"))
        dram = ctx.enter_context(tc.tile_pool(name="dram", bufs=1, space="DRAM"))

        # ---- constants ----
        ident = consts.tile([128, 128], BF16)
        make_identity(nc, ident)
        iota_pu = consts.tile([128, 1], U32)
        nc.gpsimd.iota(iota_pu, pattern=[[0, 1]], base=0, channel_multiplier=1)
        iota_p = consts.tile([128, 1], F32)
        nc.vector.tensor_copy(iota_p, iota_pu)
        s2pad = consts.tile([32, 32], F32)
        nc.vector.memset(s2pad, 0.0)

        # X broadcast: [1, D] -> [128, 2, D] fp32 (DMA with repeated reads)
        xbf2 = consts.tile([128, 2 * D], F32)
        nc.sync.dma_start(xbf2[:, 0:D], x_in[0:1, :].to_broadcast([128, D]))
        nc.sync.dma_start(xbf2[:, D:2 * D],
                          x_in[0:1, :].to_broadcast([128, D]))
        xbf = xbf2[:, 0:D]
        xbb = consts.tile([128, 2 * D], BF16)
        nc.vector.tensor_copy(xbb, xbf2)

        # ---- persistent buffers ----
        stash = big.tile([128, NT, 4, 128], BF16)   # (t - X)^T bf16
        d0buf = big.tile([128, NT], F32)            # ||t - X||^2
        sbufC = big.tile([128, NT, K], F32)

        # ================= PHASE A =================
        # pairs of row-tiles per iteration to halve instruction counts
        for g in range(NT // 2):
            tl = ld.tile([128, 2 * D], BF16, tag="tl")
            nc.sync.dma_start(
                tl[:].rearrange("p (j d) -> p j d", j=2, d=D),
                tp[g * 256:(g + 1) * 256, :].rearrange(
                    "(j p) d -> p j d", j=2, p=128))
            ub = bfp.tile([128, 2 * D], BF16, tag="ub")
            seng = nc.vector if g % 9 < 4 else nc.gpsimd
            seng.tensor_tensor(out=ub, in0=tl, in1=xbb, op=ALU.subtract)
            scr = scrp.tile([128, 2 * D], BF16, tag="scr")
            ps = psum_t.tile([128, 2, 4, 128], BF16, tag="ps")
            for j in range(2):
                f = 2 * g + j
                nc.scalar.activation(scr[:, j * D:(j + 1) * D],
                                     ub[:, j * D:(j + 1) * D], AF.Square,
                                     accum_out=d0buf[:, f:f + 1])
                for c in range(4):
                    nc.tensor.transpose(
                        ps[:, j, c, :],
                        ub[:, (j * 4 + c) * 128:(j * 4 + c + 1) * 128],
                        ident)
            nc.vector.tensor_copy(
                stash[:, 2 * g:2 * g + 2].rearrange("p a b c -> p (a b c)"),
                ps[:].rearrange("p a b c -> p (a b c)"))

        # selection score selq = -d0^2
        selq = small.tile([128, NT], F32)
        nc.vector.tensor_scalar_mul(selq, d0buf, -1.0)
        if debug:
            nc.sync.dma_start(dbg_d0[:, :], d0buf)

        if stage < 2:
            nc.sync.dma_start(out[:, :], selq[0:1, 0:1])
            return nc
        # ================= PHASE B =================
        # per-partition top-8 + local n index
        qv8 = small.tile([128, 8], F32)
        qi8 = small.tile([128, 8], U32)
        nc.vector.max_with_indices(out_max=qv8, out_indices=qi8, in_=selq)
        qn8f = small.tile([128, 8], F32)
        nc.vector.tensor_copy(qn8f, qi8)
        nc.vector.tensor_scalar(out=qn8f, in0=qn8f, scalar1=128.0,
                                scalar2=iota_p[:, 0:1], op0=ALU.mult,
                                op1=ALU.add)
        qn8 = small.tile([128, 8], U32)
        nc.vector.tensor_copy(qn8, qn8f)
        # rearrange to one partition + DRAM table of n-indices
        qv1k = small.tile([1, 1024], F32)
        nc.sync.dma_start(qv1k, qv8)
        qn_dram = dram.tile([128, 8], U32)
        nc.sync.dma_start(qn_dram, qn8)
        # local top-24 by approx score
        qv24 = small.tile([1, 24], F32)
        qpos32 = small.tile([32, 32], U32)
        _rounds_topk_vi(nc, qv1k, qv24, qpos32[0:1, 0:24])
        # positions -> partitions via DVE 32x32 stream transpose
        qposT = small.tile([32, 32], U32)
        nc.vector.transpose(qposT, qpos32)
        nl24 = small.tile([24, 1], U32)
        nc.gpsimd.indirect_dma_start(
            out=nl24, out_offset=None,
            in_=qn_dram[:].rearrange("p (j o) -> (p j) o", j=8, o=1),
            in_offset=bass.IndirectOffsetOnAxis(ap=qposT[0:24, 0:1], axis=0))
        cand24 = small.tile([24, D], F32)
        nc.gpsimd.indirect_dma_start(
            out=cand24, out_offset=None, in_=tpf[:, :],
            in_offset=bass.IndirectOffsetOnAxis(ap=nl24[:, 0:1], axis=0))
        if debug:
            nc.sync.dma_start(dbg_nl24[:, :], nl24)
        # u24 = cand - X (exact f32) and exact d0^2 for the 24 candidates
        u24 = small.tile([24, D], F32)
        nc.vector.tensor_tensor(out=u24, in0=cand24, in1=xbf[0:24, :],
                                op=ALU.subtract)
        scr24 = small.tile([24, D], BF16)
        d0x24 = small.tile([24, 1], F32)
        nc.scalar.activation(scr24, u24, AF.Square, accum_out=d0x24)
        # preload the Sqrt ACT table; runs during the collective wait
        dum = small.tile([1, 1], F32)
        nc.scalar.activation(dum, d0x24[0:1, 0:1], AF.Sqrt)
        if debug:
            nc.sync.dma_start(dbg_d0x[:, :], d0x24)
        if stage < 3:
            nc.sync.dma_start(out[:, :], d0x24[0:1, 0:1])
            return nc
        # allgather candidates: [24, D+1] = (nb - X) | exact d0^2
        cc_in = dram.tile([24, D + 1], F32)
        nc.sync.dma_start(cc_in[:, 0:D], u24)
        nc.sync.dma_start(cc_in[:, D:D + 1], d0x24)
        gath = dram.tile([NC_N * 24, D + 1], F32, addr_space="Shared")
        nc.gpsimd.collective_compute(
            "AllGather", ALU.bypass,
            replica_groups=[list(range(NC_N))],
            ins=[cc_in.opt()], outs=[gath.opt()])
        # merge: top-20 by exact d0^2 (negated -> max)
        gv = small.tile([1, NC_N * 24], F32)
        nc.sync.dma_start(gv, gath[:, D:D + 1])
        nc.vector.tensor_scalar_mul(gv, gv, -1.0)
        gv24 = small.tile([1, 24], F32)
        gpos32 = small.tile([32, 32], U32)
        _rounds_topk_vi(nc, gv, gv24, gpos32[0:1, 0:24])
        # pdist_x^2 * 20 = -(sum of top-20 negated d0^2)
        sd0 = small.tile([1, 1], F32)
        nc.vector.tensor_reduce(out=sd0, in_=gv24[:, 0:K],
                                axis=mybir.AxisListType.X, op=ALU.add)
        # gather the 20 neighbor rows (u = nb - X | d0x)
        gposT = small.tile([32, 32], U32)
        nc.vector.transpose(gposT, gpos32)
        nbrow = small.tile([K, D + 1], F32)
        nc.gpsimd.indirect_dma_start(
            out=nbrow, out_offset=None, in_=gath[:, :],
            in_offset=bass.IndirectOffsetOnAxis(ap=gposT[0:K, 0:1], axis=0))
        if debug:
            nc.sync.dma_start(dbg_nb[:, :], nbrow[:, 0:D])
        # nbT = 2*(nb - X) in bf16: [128, 4, K]
        nbb = small.tile([K, D], BF16)
        nc.vector.tensor_scalar_mul(nbb, nbrow[:, 0:D], 2.0)
        psn = psum_s.tile([128, 4, K], BF16)
        for c in range(4):
            nc.tensor.transpose(psn[:, c, :], nbb[:, c * 128:(c + 1) * 128],
                                ident[0:K, 0:K])
        nbT = small.tile([128, 4, K], BF16)
        nc.vector.tensor_copy(nbT, psn)

        if stage < 4:
            nc.gpsimd.dma_start(out[:, :], nbT[0:1, 0, 0:1])
            return nc
        # ================= PHASE C =================
        G4 = 4
        ngr = (NT + G4 - 1) // G4
        for g in range(ngr):
            f0 = g * G4
            gn = min(G4, NT - f0)
            psc = psum_c.tile([128, G4, K], F32, tag="psc")
            for j in range(gn):
                for c in range(4):
                    nc.tensor.matmul(psc[:, j, :], lhsT=stash[:, f0 + j, c, :],
                                     rhs=nbT[:, c, :],
                                     start=(c == 0), stop=(c == 3))
            # s = 2*u.unb - d0^2, batched over the tile group
            nc.vector.scalar_tensor_tensor(
                out=sbufC[:, f0:f0 + gn, :], in0=psc[:, 0:gn, :], scalar=0.0,
                in1=d0buf[:, f0:f0 + gn].to_broadcast([128, gn, K]),
                op0=ALU.bypass, op1=ALU.subtract)

        # per-row top-8 per partition (values only; larger s = nearer)
        cv8 = small.tile([128, 24, 8], F32)
        for k in range(K):
            nc.vector.max(out=cv8[:, k, :], in_=sbufC[:, :, k])
        # collapse partitions on-chip: 32x32 stream transposes put the
        # (k, top-slot) pairs on partitions and source partitions on the
        # free axis, so both DMA bounce sides are contiguous.
        cvT = small.tile([96, 128], F32)
        for b in range(4):
            for c in range(3):
                nc.vector.transpose(
                    cvT[32 * c:32 * c + 32, 32 * b:32 * b + 32],
                    cv8[32 * b:32 * b + 32, 8 * c:8 * c + 8, 0:4])
        cvdT = dram.tile([K * 4, 128], F32)
        nc.sync.dma_start(cvdT, cvT[0:K * 4, :])
        cvM = small.tile([K, 512], F32)
        nc.sync.dma_start(
            cvM, cvdT[:].rearrange("(k j) p -> k (j p)", k=K, j=4))
        if stage < 5:
            nc.sync.dma_start(out[:, :], cvM[0:1, 0:1])
            return nc
        # local top-24 s values per row, keep top-20
        cv24 = small.tile([K, 24], F32)
        _rounds_topk_v(nc, cvM, cv24)
        # allgather local top-20 s values -> [8*K, K]
        c2_in = dram.tile([K, K], F32)
        nc.sync.dma_start(c2_in, cv24[:, 0:K])
        gath2 = dram.tile([NC_N * K, K], F32, addr_space="Shared")
        nc.gpsimd.collective_compute(
            "AllGather", ALU.bypass,
            replica_groups=[list(range(NC_N))],
            ins=[c2_in.opt()], outs=[gath2.opt()])
        # merge per row: [K, 8*K] -> global top-20 s
        g2 = small.tile([K, NC_N * K], F32)
        nc.sync.dma_start(
            g2, gath2[:].rearrange("(j k) m -> k j m", j=NC_N, k=K))
        g2v = small.tile([K, 24], F32)
        _rounds_topk_v(nc, g2, g2v)
        # S2[k] = mean top-20 dist^2 = d0x_k - (sum of top-20 s)/20
        sumS = small.tile([K, 1], F32)
        nc.vector.tensor_reduce(out=sumS, in_=g2v[:, 0:K],
                                axis=mybir.AxisListType.X, op=ALU.add)
        nc.vector.tensor_scalar(out=s2pad[0:K, 0:1], in0=sumS,
                                scalar1=-1.0 / K, scalar2=nbrow[:, D:D + 1],
                                op0=ALU.mult, op1=ALU.add)
        if debug:
            nc.sync.dma_start(dbg_s2[:, :], s2pad[0:K, 0:1])

        if stage < 7:
            nc.sync.dma_start(out[:, :], s2pad[0:1, 0:1])
            return nc
        # ================= PHASE D =================
        # norm_factor = sum sqrt(s2): stream-transpose s2 to one row, then
        # a single ACT sqrt with free-axis accumulate (table preloaded).
        s2row = small.tile([32, 32], F32)
        nc.vector.transpose(s2row, s2pad)
        sq20 = small.tile([1, K], F32)
        nf = small.tile([1, 1], F32)
        nc.scalar.activation(sq20, s2row[0:1, 0:K], AF.Sqrt, accum_out=nf)
        # pdist_x = sqrt(-sd0/20)
        px = small.tile([1, 1], F32)
        nc.scalar.activation(px, sd0, AF.Sqrt, scale=-1.0 / K)
        # z = lof/sqrt(2) = (px/nf*K - 1)*SQ2I
        rnf = small.tile([1, 1], F32)
        nc.vector.reciprocal(rnf, nf)
        z = small.tile([1, 1], F32)
        nc.vector.tensor_tensor(out=z, in0=px, in1=rnf, op=ALU.mult)
        nc.vector.tensor_scalar(out=z, in0=z, scalar1=float(K) * SQ2I,
                                scalar2=-SQ2I, op0=ALU.mult, op1=ALU.add)
        # erf(z) ~= TPI*z*(1 - z^2/3 + z^4/10)  (|z| << 1 here)
        z2 = small.tile([1, 1], F32)
        nc.vector.tensor_tensor(out=z2, in0=z, in1=z, op=ALU.mult)
        ta = small.tile([1, 1], F32)
        nc.vector.tensor_scalar(out=ta, in0=z2, scalar1=-1.0 / 3.0,
                                scalar2=1.0, op0=ALU.mult, op1=ALU.add)
        tb = small.tile([1, 1], F32)
        nc.vector.tensor_tensor(out=tb, in0=z2, in1=z2, op=ALU.mult)
        tcp = small.tile([1, 1], F32)
        nc.vector.scalar_tensor_tensor(out=tcp, in0=tb, scalar=0.1, in1=ta,
                                       op0=ALU.mult, op1=ALU.add)
        te = small.tile([1, 1], F32)
        nc.vector.tensor_tensor(out=te, in0=z, in1=tcp, op=ALU.mult)
        res = small.tile([1, 1], F32)
        nc.vector.tensor_scalar(out=res, in0=te, scalar1=TPI, scalar2=0.0,
                                op0=ALU.mult, op1=ALU.max)
        nc.sync.dma_start(out[:, :], res)

    return nc


def prepare_inputs(X, train_points):
    """Pad + shard the full inputs into per-core in_maps.

    Ships a bf16 copy of the shard (phase A streams it; halves HBM
    traffic) plus the f32 shard for the exact 24-row rescore gather.
    """
    import ml_dtypes

    X = np.ascontiguousarray(X, dtype=np.float32)
    tpts = np.ascontiguousarray(train_points, dtype=np.float32)
    n = tpts.shape[0]
    pad = np.full((NPAD - n, D), PADV, dtype=np.float32)
    tpad = np.concatenate([tpts, pad], axis=0)
    tpad_bf = tpad.astype(ml_dtypes.bfloat16)
    in_maps = []
    for i in range(NC_N):
        in_maps.append({
            "tp": np.ascontiguousarray(tpad_bf[i * NLOC:(i + 1) * NLOC]),
            "tpf": np.ascontiguousarray(tpad[i * NLOC:(i + 1) * NLOC]),
            "x": X.reshape(1, D),
        })
    return in_maps


_NC_CACHE = {}


def kernel(X, train_points):
    from concourse.bass_utils import run_bass_kernel_spmd

    if "nc" not in _NC_CACHE:
        _NC_CACHE["nc"] = build(debug=False)
    nc = _NC_CACHE["nc"]
    in_maps = prepare_inputs(X, train_points)
    res = run_bass_kernel_spmd(nc, in_maps, list(range(NC_N)), trace=False)
    out = np.asarray(res.results[0]["out"], dtype=np.float32).reshape(())
    return out


# revision 19
# speedup vs baseline: 1.1660x; 1.1660x over previous
"""LoOP (Local Outlier Probability) kernel for 8 TRN2 NeuronCores.

kernel(X, train_points) computes the reference nn_LoOP forward pass:
brute-force 20-NN of X over train_points, the 20-NN of each neighbor,
pdist ratios, and max(erf(lof/sqrt(2)), 0) -- distributed over 8 cores
(row-sharded train_points), with all compute on-device.

Distance trick: with u_t = t - X stashed transposed (bf16) and
unb_k = nb_k - X, the score s = 2*u_t.unb_k - ||u_t||^2 equals
||nb_k - X||^2 - ||t - nb_k||^2, so per-neighbor nearest rows are
argmax(s) and exact-enough distances are d0x_k - s (d0x_k exact f32).
No per-candidate gather/re-square is needed for the second kNN.
"""

import sys
import types
from contextlib import ExitStack

import numpy as np

import bass_rust
import concourse.bass as bass
import concourse.mybir as mybir
import concourse.tile as tile
from concourse.masks import make_identity
from concourse.tile import TileContext
from concourse.vector_clock import ScopedClock


# ---------------------------------------------------------------------------
# Toolchain workarounds: this walrus build accepts at most ONE sync wait per
# instruction (two for EventSemaphore), and the Tile kernel-tail drain
# collects one wait per outstanding sem domain. Split both.
# ---------------------------------------------------------------------------
def _split_multi_waits(nc):
    """This walrus build accepts at most ONE sync wait per instruction
    (two for EventSemaphore). Tile attaches as many waits as deps require.
    Rewrite: keep the first wait on the instruction, hoist extras onto
    same-engine NOPs inserted immediately before it."""
    edits = []
    for f in nc.m.functions:
        for bb in f.blocks:
            edits.append((bb, list(bb.instructions)))
    new_lists = []
    for bb, insts in edits:
        new = []
        changed = False
        for inst in insts:
            si = inst.sync_info
            cap = 2 if isinstance(inst, bass_rust.InstEventSemaphore) else 1
            if si is not None and si.on_wait and len(si.on_wait) > cap:
                waits = list(si.on_wait)
                for w in waits[cap:]:
                    nop = nc.engines[inst.engine].nop(nofuse=True).ins
                    nop.sync_info = bass_rust.SyncInfo(on_wait=[w],
                                                       on_update=[])
                    new.append(nop)
                inst.sync_info = bass_rust.SyncInfo(
                    on_wait=waits[:cap], on_update=list(si.on_update or []))
                changed = True
            new.append(inst)
        new_lists.append((bb, new, changed))
    for bb, new, changed in new_lists:
        if changed:
            bb.instructions = new


def _patched_drain_and_barrier(self, tick_clock, wait_clock):
    nc = self.nc
    _split_multi_waits(nc)
    drain_inst = nc.sync.drain()
    wait_clock.add_sem_waits(
        drain_inst.ins, ScopedClock({None: tick_clock.global_clock})
    )
    si = drain_inst.ins.sync_info
    if si is not None and si.on_wait and len(si.on_wait) > 1:
        waits = list(si.on_wait)
        upd = list(si.on_update or [])
        drain_inst.ins.sync_info = bass_rust.SyncInfo(
            on_wait=[waits[0]], on_update=upd
        )
        for w in waits[1:]:
            extra = nc.sync.drain()
            extra.ins.sync_info = bass_rust.SyncInfo(on_wait=[w], on_update=[])

    nc.all_engine_barrier()
    assert self.sems is not None
    popped = nc._tile_sem_poison_stack.pop()
    assert popped is self._sem_poison
    nc.clear_and_free_semaphores(list(self.sems.allocated().values()))
    nc.all_engine_barrier()


def install():
    TileContext._drain_and_barrier = _patched_drain_and_barrier
    try:
        _install_ntff_hook()
    except Exception:
        pass  # profiling hook is optional


def _install_ntff_hook():
    if "antenv.axon_hooks" in sys.modules:
        return
    mod = types.ModuleType("antenv.axon_hooks")
    state = {"hook": None}
    mod.set_axon_ntff_profile_hook = lambda h: state.__setitem__("hook", h)
    mod.get_axon_ntff_profile_hook = lambda: state["hook"]
    sys.modules["antenv.axon_hooks"] = mod
    import antenv

    antenv.axon_hooks = mod
    from trn_agent_boot.trn_boot import _ntff_profile_via_ctypes

    hook = _ntff_profile_via_ctypes("/opt/axon/libaxon_pjrt.so")
    if hook is not None:
        mod.set_axon_ntff_profile_hook(hook)




install()


F32 = mybir.dt.float32
BF16 = mybir.dt.bfloat16
U32 = mybir.dt.uint32
AF = mybir.ActivationFunctionType
ALU = mybir.AluOpType

NC_N = 8          # cores
D = 512           # feature dim
K = 20            # neighbors
NT = 98           # tiles per core
NLOC = NT * 128   # 12544 rows per core (padded)
NPAD = NC_N * NLOC
PADV = 1.0e4      # padding row fill value
NEG = -3.0e38

SQ2I = 0.7071067811865476
TPI = 1.1283791670955126  # 2/sqrt(pi)


def _rounds_topk_vi(nc, work, vals, pos, n_rounds=3):
    """max/match_replace rounds on `work` [P, F]; writes descending values
    into vals [P, 8*n] and positions into pos (uint32)."""
    for r in range(n_rounds):
        v8 = vals[:, 8 * r:8 * r + 8]
        nc.vector.max_with_indices(out_max=v8,
                                   out_indices=pos[:, 8 * r:8 * r + 8],
                                   in_=work)
        if r < n_rounds - 1:
            nc.vector.match_replace(out=work, in_to_replace=v8,
                                    in_values=work, imm_value=NEG)


def _rounds_topk_v(nc, work, vals, n_rounds=3):
    """values-only top-8*n rounds (no index extraction)."""
    for r in range(n_rounds):
        v8 = vals[:, 8 * r:8 * r + 8]
        nc.vector.max(out=v8, in_=work)
        if r < n_rounds - 1:
            nc.vector.match_replace(out=work, in_to_replace=v8,
                                    in_values=work, imm_value=NEG)


def build(debug=False, stage=99):
    nc = bass.Bass()
    tp = nc.declare_dram_parameter("tp", [NLOC, D], BF16, isOutput=False)
    x_in = nc.declare_dram_parameter("x", [1, D], F32, isOutput=False)
    out = nc.declare_dram_parameter("out", [1, 1], F32, isOutput=True)
    if debug:
        dbg_d0 = nc.declare_dram_parameter("dbg_d0", [128, NT], F32, isOutput=True)
        dbg_nl24 = nc.declare_dram_parameter("dbg_nl24", [24, 1], U32, isOutput=True)
        dbg_d0x = nc.declare_dram_parameter("dbg_d0x", [24, 1], F32, isOutput=True)
        dbg_nb = nc.declare_dram_parameter("dbg_nb", [K, D], F32, isOutput=True)
        dbg_s2 = nc.declare_dram_parameter("dbg_s2", [K, 1], F32, isOutput=True)

    with tile.TileContext(nc) as tc, ExitStack() as ctx:
        # ---- pools ----
        consts = ctx.enter_context(tc.tile_pool(name="consts", bufs=1))
        big = ctx.enter_context(tc.tile_pool(name="big", bufs=1))
        ld = ctx.enter_context(tc.tile_pool(name="ld", bufs=6))
        bfp = ctx.enter_context(tc.tile_pool(name="bfp", bufs=4))
        scrp = ctx.enter_context(tc.tile_pool(name="scrp", bufs=3))
        small = ctx.enter_context(tc.tile_pool(name="small", bufs=1))
        psum_t = ctx.enter_context(tc.tile_pool(name="psum_t", bufs=3, space="PSUM"))
        psum_c = ctx.enter_context(tc.tile_pool(name="psum_c", bufs=3, space="PSUM"))
        psum_s = ctx.enter_context(tc.tile_pool(name="psum_s", bufs=1, space="PSUM"))
        dram = ctx.enter_context(tc.tile_pool(name="dram", bufs=1, space="DRAM"))

        # ---- constants ----
        ident = consts.tile([128, 128], BF16)
        make_identity(nc, ident)
        iota_pu = consts.tile([128, 1], U32)
        nc.gpsimd.iota(iota_pu, pattern=[[0, 1]], base=0, channel_multiplier=1)
        iota_p = consts.tile([128, 1], F32)
        nc.vector.tensor_copy(iota_p, iota_pu)
        s2pad = consts.tile([32, 32], F32)
        nc.vector.memset(s2pad, 0.0)
        dumsq = consts.tile([128, 1], F32)

        # X broadcast: [1, D] -> [128, 2, D] fp32 (DMA with repeated reads)
        xbf2 = consts.tile([128, 2 * D], F32)
        nc.sync.dma_start(xbf2[:, 0:D], x_in[0:1, :].to_broadcast([128, D]))
        nc.sync.dma_start(xbf2[:, D:2 * D],
                          x_in[0:1, :].to_broadcast([128, D]))
        xbf = xbf2[:, 0:D]
        xbb = consts.tile([128, 2 * D], BF16)
        nc.vector.tensor_copy(xbb, xbf2)

        # ---- persistent buffers ----
        stash = big.tile([128, NT, 4, 128], BF16)   # (t - X)^T bf16
        d0buf = big.tile([128, NT], F32)            # ||t - X||^2
        sbufC = big.tile([128, NT, K], F32)

        # ================= PHASE A =================
        # pairs of row-tiles per iteration to halve instruction counts
        for g in range(NT // 2):
            tl = ld.tile([128, 2 * D], BF16, tag="tl")
            nc.sync.dma_start(
                tl[:].rearrange("p (j d) -> p j d", j=2, d=D),
                tp[g * 256:(g + 1) * 256, :].rearrange(
                    "(j p) d -> p j d", j=2, p=128))
            ub = bfp.tile([128, 2 * D], BF16, tag="ub")
            seng = nc.vector if g % 9 < 4 else nc.gpsimd
            seng.tensor_tensor(out=ub, in0=tl, in1=xbb, op=ALU.subtract)
            scr = scrp.tile([128, 2 * D], BF16, tag="scr")
            ps = psum_t.tile([128, 2, 4, 128], BF16, tag="ps")
            for j in range(2):
                f = 2 * g + j
                ubj = ub[:, j * D:(j + 1) * D]
                nc.scalar.activation(scr[:, j * D:(j + 1) * D],
                                     ubj, AF.Square,
                                     accum_out=d0buf[:, f:f + 1])
                for c in range(4):
                    nc.tensor.transpose(
                        ps[:, j, c, :],
                        ub[:, (j * 4 + c) * 128:(j * 4 + c + 1) * 128],
                        ident)
            nc.vector.tensor_copy(
                stash[:, 2 * g:2 * g + 2].rearrange("p a b c -> p (a b c)"),
                ps[:].rearrange("p a b c -> p (a b c)"))

        # selection score selq = -d0^2
        selq = small.tile([128, NT], F32)
        nc.vector.tensor_scalar_mul(selq, d0buf, -1.0)
        if debug:
            nc.sync.dma_start(dbg_d0[:, :], d0buf)

        if stage < 2:
            nc.sync.dma_start(out[:, :], selq[0:1, 0:1])
            return nc
        # ================= PHASE B =================
        # per-partition top-8 + local n index
        qv8 = small.tile([128, 8], F32)
        qi8 = small.tile([128, 8], U32)
        nc.vector.max_with_indices(out_max=qv8, out_indices=qi8, in_=selq)
        qn8f = small.tile([128, 8], F32)
        nc.vector.tensor_copy(qn8f, qi8)
        nc.vector.tensor_scalar(out=qn8f, in0=qn8f, scalar1=128.0,
                                scalar2=iota_p[:, 0:1], op0=ALU.mult,
                                op1=ALU.add)
        qn8 = small.tile([128, 8], U32)
        nc.vector.tensor_copy(qn8, qn8f)
        # rearrange to one partition + DRAM table of n-indices
        qv1k = small.tile([1, 1024], F32)
        nc.sync.dma_start(qv1k, qv8)
        qn_dram = dram.tile([128, 8], U32)
        nc.sync.dma_start(qn_dram, qn8)
        # local top-24 by approx score (values are -d0^2, kept consistent
        # with the matmul-phase scores; no exact rescore needed)
        qval32 = small.tile([32, 32], F32)
        qpos32 = small.tile([32, 32], U32)
        _rounds_topk_vi(nc, qv1k, qval32[0:1, 0:24], qpos32[0:1, 0:24])
        # positions/values -> partitions via DVE 32x32 stream transpose
        qposT = small.tile([32, 32], U32)
        nc.vector.transpose(qposT, qpos32)
        qvalT = small.tile([32, 32], F32)
        nc.vector.transpose(qvalT, qval32)
        d0x24 = small.tile([24, 1], F32)
        nc.vector.tensor_scalar_mul(d0x24, qvalT[0:24, 0:1], -1.0)
        nl24 = small.tile([24, 1], U32)
        nc.gpsimd.indirect_dma_start(
            out=nl24, out_offset=None,
            in_=qn_dram[:].rearrange("p (j o) -> (p j) o", j=8, o=1),
            in_offset=bass.IndirectOffsetOnAxis(ap=qposT[0:24, 0:1], axis=0))
        cand24 = small.tile([24, D], BF16)
        nc.gpsimd.indirect_dma_start(
            out=cand24, out_offset=None, in_=tp[:, :],
            in_offset=bass.IndirectOffsetOnAxis(ap=nl24[:, 0:1], axis=0))
        if debug:
            nc.sync.dma_start(dbg_nl24[:, :], nl24)
        # u24 = cand - X in f32
        u24 = small.tile([24, D], F32)
        nc.vector.tensor_tensor(out=u24, in0=cand24, in1=xbf[0:24, :],
                                op=ALU.subtract)
        # preload the Sqrt ACT table; runs during the collective wait
        dum = small.tile([1, 1], F32)
        nc.scalar.activation(dum, d0buf[0:1, 0:1], AF.Sqrt)
        if debug:
            nc.sync.dma_start(dbg_d0x[:, :], d0x24)
        if stage < 3:
            nc.sync.dma_start(out[:, :], d0x24[0:1, 0:1])
            return nc
        # allgather candidates: [24, D+1] = (nb - X) | exact d0^2
        cc_in = dram.tile([24, D + 1], F32)
        nc.sync.dma_start(cc_in[:, 0:D], u24)
        nc.sync.dma_start(cc_in[:, D:D + 1], d0x24)
        gath = dram.tile([NC_N * 24, D + 1], F32, addr_space="Shared")
        nc.gpsimd.collective_compute(
            "AllGather", ALU.bypass,
            replica_groups=[list(range(NC_N))],
            ins=[cc_in.opt()], outs=[gath.opt()])
        # merge: top-20 by exact d0^2 (negated -> max)
        gv = small.tile([1, NC_N * 24], F32)
        nc.sync.dma_start(gv, gath[:, D:D + 1])
        nc.vector.tensor_scalar_mul(gv, gv, -1.0)
        gv24 = small.tile([1, 24], F32)
        gpos32 = small.tile([32, 32], U32)
        _rounds_topk_vi(nc, gv, gv24, gpos32[0:1, 0:24])
        # pdist_x^2 * 20 = -(sum of top-20 negated d0^2)
        sd0 = small.tile([1, 1], F32)
        nc.vector.tensor_reduce(out=sd0, in_=gv24[:, 0:K],
                                axis=mybir.AxisListType.X, op=ALU.add)
        # gather the 20 neighbor rows (u = nb - X | d0x)
        gposT = small.tile([32, 32], U32)
        nc.vector.transpose(gposT, gpos32)
        nbrow = small.tile([K, D + 1], F32)
        nc.gpsimd.indirect_dma_start(
            out=nbrow, out_offset=None, in_=gath[:, :],
            in_offset=bass.IndirectOffsetOnAxis(ap=gposT[0:K, 0:1], axis=0))
        if debug:
            nc.sync.dma_start(dbg_nb[:, :], nbrow[:, 0:D])
        # nbT = 2*(nb - X) in bf16: [128, 4, K]
        nbb = small.tile([K, D], BF16)
        nc.vector.tensor_scalar_mul(nbb, nbrow[:, 0:D], 2.0)
        psn = psum_s.tile([128, 4, K], BF16)
        for c in range(4):
            nc.tensor.transpose(psn[:, c, :], nbb[:, c * 128:(c + 1) * 128],
                                ident[0:K, 0:K])
        nbT = small.tile([128, 4, K], BF16)
        nc.vector.tensor_copy(nbT, psn)

        if stage < 4:
            nc.gpsimd.dma_start(out[:, :], nbT[0:1, 0, 0:1])
            return nc
        # ================= PHASE C =================
        G4 = 4
        ngr = (NT + G4 - 1) // G4
        for g in range(ngr):
            f0 = g * G4
            gn = min(G4, NT - f0)
            psc = psum_c.tile([128, G4, K], F32, tag="psc")
            for j in range(gn):
                for c in range(4):
                    nc.tensor.matmul(psc[:, j, :], lhsT=stash[:, f0 + j, c, :],
                                     rhs=nbT[:, c, :],
                                     start=(c == 0), stop=(c == 3))
            # s = 2*u.unb - d0^2, batched over the tile group
            nc.vector.scalar_tensor_tensor(
                out=sbufC[:, f0:f0 + gn, :], in0=psc[:, 0:gn, :], scalar=0.0,
                in1=d0buf[:, f0:f0 + gn].to_broadcast([128, gn, K]),
                op0=ALU.bypass, op1=ALU.subtract)

        # per-row top-8 per partition (values only; larger s = nearer)
        cv8 = small.tile([128, 24, 8], F32)
        for k in range(K):
            nc.vector.max(out=cv8[:, k, :], in_=sbufC[:, :, k])
        # collapse partitions on-chip: 32x32 stream transposes put the
        # (k, top-slot) pairs on partitions and source partitions on the
        # free axis, so both DMA bounce sides are contiguous.
        cvT = small.tile([96, 128], F32)
        for b in range(4):
            for c in range(3):
                nc.vector.transpose(
                    cvT[32 * c:32 * c + 32, 32 * b:32 * b + 32],
                    cv8[32 * b:32 * b + 32, 8 * c:8 * c + 8, 0:4])
        cvdT = dram.tile([K * 4, 128], F32)
        nc.sync.dma_start(cvdT, cvT[0:K * 4, :])
        cvM = small.tile([K, 512], F32)
        nc.sync.dma_start(
            cvM, cvdT[:].rearrange("(k j) p -> k (j p)", k=K, j=4))
        if stage < 5:
            nc.sync.dma_start(out[:, :], cvM[0:1, 0:1])
            return nc
        # local top-24 s values per row, keep top-20
        cv24 = small.tile([K, 24], F32)
        _rounds_topk_v(nc, cvM, cv24)
        # allgather local top-20 s values -> [8*K, K]
        c2_in = dram.tile([K, K], F32)
        nc.sync.dma_start(c2_in, cv24[:, 0:K])
        gath2 = dram.tile([NC_N * K, K], F32, addr_space="Shared")
        nc.gpsimd.collective_compute(
            "AllGather", ALU.bypass,
            replica_groups=[list(range(NC_N))],
            ins=[c2_in.opt()], outs=[gath2.opt()])
        # merge per row: [K, 8*K] -> global top-20 s
        g2 = small.tile([K, NC_N * K], F32)
        nc.sync.dma_start(
            g2, gath2[:].rearrange("(j k) m -> k j m", j=NC_N, k=K))
        g2v = small.tile([K, 24], F32)
        _rounds_topk_v(nc, g2, g2v)
        # S2[k] = mean top-20 dist^2 = d0x_k - (sum of top-20 s)/20
        sumS = small.tile([K, 1], F32)
        nc.vector.tensor_reduce(out=sumS, in_=g2v[:, 0:K],
                                axis=mybir.AxisListType.X, op=ALU.add)
        nc.vector.tensor_scalar(out=s2pad[0:K, 0:1], in0=sumS,
                                scalar1=-1.0 / K, scalar2=nbrow[:, D:D + 1],
                                op0=ALU.mult, op1=ALU.add)
        if debug:
            nc.sync.dma_start(dbg_s2[:, :], s2pad[0:K, 0:1])

        if stage < 7:
            nc.sync.dma_start(out[:, :], s2pad[0:1, 0:1])
            return nc
        # ================= PHASE D =================
        # norm_factor = sum sqrt(s2): stream-transpose s2 to one row, then
        # a single ACT sqrt with free-axis accumulate (table preloaded).
        s2row = small.tile([32, 32], F32)
        nc.vector.transpose(s2row, s2pad)
        sq20 = small.tile([1, K], F32)
        nf = small.tile([1, 1], F32)
        nc.scalar.activation(sq20, s2row[0:1, 0:K], AF.Sqrt, accum_out=nf)
        # pdist_x = sqrt(-sd0/20)
        px = small.tile([1, 1], F32)
        nc.scalar.activation(px, sd0, AF.Sqrt, scale=-1.0 / K)
        # z = lof/sqrt(2) = (px/nf*K - 1)*SQ2I
        rnf = small.tile([1, 1], F32)
        nc.vector.reciprocal(rnf, nf)
        z = small.tile([1, 1], F32)
        nc.vector.tensor_tensor(out=z, in0=px, in1=rnf, op=ALU.mult)
        nc.vector.tensor_scalar(out=z, in0=z, scalar1=float(K) * SQ2I,
                                scalar2=-SQ2I, op0=ALU.mult, op1=ALU.add)
        # erf(z) ~= TPI*z*(1 - z^2/3 + z^4/10)  (|z| << 1 here)
        z2 = small.tile([1, 1], F32)
        nc.vector.tensor_tensor(out=z2, in0=z, in1=z, op=ALU.mult)
        ta = small.tile([1, 1], F32)
        nc.vector.tensor_scalar(out=ta, in0=z2, scalar1=-1.0 / 3.0,
                                scalar2=1.0, op0=ALU.mult, op1=ALU.add)
        tb = small.tile([1, 1], F32)
        nc.vector.tensor_tensor(out=tb, in0=z2, in1=z2, op=ALU.mult)
        tcp = small.tile([1, 1], F32)
        nc.vector.scalar_tensor_tensor(out=tcp, in0=tb, scalar=0.1, in1=ta,
                                       op0=ALU.mult, op1=ALU.add)
        te = small.tile([1, 1], F32)
        nc.vector.tensor_tensor(out=te, in0=z, in1=tcp, op=ALU.mult)
        res = small.tile([1, 1], F32)
        nc.vector.tensor_scalar(out=res, in0=te, scalar1=TPI, scalar2=0.0,
                                op0=ALU.mult, op1=ALU.max)
        nc.sync.dma_start(out[:, :], res)

    return nc


def prepare_inputs(X, train_points):
    """Pad + shard the full inputs into per-core in_maps.

    Ships a bf16 copy of the shard (phase A streams it; halves HBM
    traffic) plus the f32 shard for the exact 24-row rescore gather.
    """
    import ml_dtypes

    X = np.ascontiguousarray(X, dtype=np.float32)
    tpts = np.ascontiguousarray(train_points, dtype=np.float32)
    n = tpts.shape[0]
    pad = np.full((NPAD - n, D), PADV, dtype=np.float32)
    tpad = np.concatenate([tpts, pad], axis=0)
    tpad_bf = tpad.astype(ml_dtypes.bfloat16)
    in_maps = []
    for i in range(NC_N):
        in_maps.append({
            "tp": np.ascontiguousarray(tpad_bf[i * NLOC:(i + 1) * NLOC]),
            "x": X.reshape(1, D),
        })
    return in_maps


_NC_CACHE = {}


def kernel(X, train_points):
    from concourse.bass_utils import run_bass_kernel_spmd

    if "nc" not in _NC_CACHE:
        _NC_CACHE["nc"] = build(debug=False)
    nc = _NC_CACHE["nc"]
    in_maps = prepare_inputs(X, train_points)
    res = run_bass_kernel_spmd(nc, in_maps, list(range(NC_N)), trace=False)
    out = np.asarray(res.results[0]["out"], dtype=np.float32).reshape(())
    return out
